# revision 43
# baseline (speedup 1.0000x reference)
"""Masked 3-layer MLP (tanh) on 8 Trainium2 NeuronCores.

Reference computation (B=2048, dims 4096->8192->8192->4096, fp32):
    h1 = tanh(x @ (W1*m1).T + b1)
    h2 = tanh(h1 @ (W2*m2).T + b2)
    out =      h2 @ (W3*m3).T + b3

The masks are p=1e-4 Bernoulli, so the effective network is tiny. Fast
path ("packed"): output rows are assigned to cores by a greedy set-union
clustering (rows sharing h2 features land on the same core, which hits
the theoretical minimum of ~344 used h2 features per core). Walking the
masks backwards from each core's row set: needed h2 features S3_k, then
h1 features S2_k = nonzero m2 columns over S3_k, then x dims S1_k. The
host gathers the masked weight submatrices over exactly those index sets
(zero-padded to shared multiples of 128), and each core runs a fully
LOCAL dense 3-layer MLP with contractions ~128->256->384 instead of
4096->8192->8192. No collectives, no DRAM intermediates: weights, the
x-pack and both hidden activations stay SBUF-resident; only the final
[512, B] shard is written out (fp8 product-only by default — the output
is ~96% bias by magnitude, so the host adds the exact fp32 bias).

Compute is in transposed orientation [features, batch]: output features
land on PSUM partitions, so the per-partition bias + descale + tanh fuse
into the PSUM eviction. Default compute dtype is fp8 e4m3 with DoubleRow
matmuls (2 K-subtiles per instruction at 2x rate); weights are host-
scaled by 128 (above e4m3's min-normal), x by 16, both undone exactly by
the eviction's power-of-two `scale`; biases stay exact fp32, which keeps
rel err ~1e-3.

The kernel is DMA-latency- and ScalarE-chain-bound, so the schedule is
built around two shared single resources (HWDGE descriptor-gen, ~625ns
per DMA, and the transfer engine) plus the serial ACT tanh chain:
  - ONE fused input tensor carries w1|w2|w3 and the fp32 biases as raw
    bytes (fp32-bitcast on device), so the input phase is one wall DMA
    plus four 512-batch x chunks, each arriving just ahead of its slot
    in the tanh chain. x/h DoubleRow zero pad planes are memset on
    device (Pool/DVE, off the critical path) instead of DMA'd.
  - L1 evicts per 512-block (chain starts after the first x chunk);
    L1 matmuls issue as two 256 halves so only the first rides the PE
    p-state ramp. L2 evicts 1024-pairs (fewer ACT ops once saturated).
  - L3 runs per-512-block psum tiles (a shared tile would WAR-stall the
    next matmul behind the previous eviction), evictions alternating
    DVE/ACT (GPSIMD cannot read PSUM), flushed as two [128,1024] DMAs.

Fallback (masks not sparse enough to pack): the previous Megatron-style
column-parallel dense fp16 kernel with on-chip AllGathers after layers
1/2.
"""

import os
import sys

import numpy as np

for _p in ("/opt/trn_rl_repo", os.path.expanduser("~/.axon_site/_ro/trn_rl_repo")):
    if os.path.isdir(_p) and _p not in sys.path:
        sys.path.append(_p)

B = 2048
DIMS = [4096, 8192, 8192, 4096]
NCORES = 8
P = 128
FD = 512           # matmul moving free dim == one PSUM bank of fp32
NB = B // FD       # batch blocks
ICK = 4            # K-subtiles (x128 rows) per streamed input chunk
MCK = 4            # K-subtiles per weight/mask load+mask chunk

# Compute dtype: fp8 | fp16 | bf16 | fp32r | fp32
DTYPE = os.environ.get("BASS_MLP_DTYPE", "fp8")
# Output mode: "fp32" | "cdt" (fp16) | "prod8" (fp8 product-only: the
# device emits h2@W3 scaled by OSCALE without bias — the output is ~96%
# bias by magnitude, so the host adds exact fp32 b3 and the fp8 product
# quantization is negligible; out DMA shrinks to 1MB/core)
OUT_DT = os.environ.get("BASS_MLP_OUT_DT", "prod8")
OSCALE = 131072.0   # 2**17; device product absmax ~4e-4 -> ~52 in e4m3
# fp8 pre-scales: weights sit near e4m3's min-normal (0.0156), so scale
# them up into the normal range; x gets a mild scale for its small tail.
# The product scale is undone exactly (power of two) by the activation's
# `scale` parameter at PSUM-eviction time. Biases stay exact fp32.
SCALE_W = 128.0
SCALE_X = 16.0

_cache = {}


def _np_cdt():
    if DTYPE in ("bf16", "fp8"):
        import ml_dtypes

        return {"bf16": ml_dtypes.bfloat16,
                "fp8": ml_dtypes.float8_e4m3}[DTYPE]
    return {"fp16": np.float16, "fp32r": np.float32, "fp32": np.float32}[DTYPE]


def _scales():
    if DTYPE == "fp8":
        return SCALE_W, SCALE_X
    return 1.0, 1.0


# The dense fallback has K up to 8192 and no per-layer rescaling; run it in
# fp16 when the packed path's fp8 dtype is selected.
def _dense_dtype():
    return "fp16" if DTYPE == "fp8" else DTYPE


def _np_dense_dt():
    if _dense_dtype() == "bf16":
        import ml_dtypes

        return ml_dtypes.bfloat16
    return {"fp16": np.float16, "fp32r": np.float32,
            "fp32": np.float32}[_dense_dtype()]


# --------------------------------------------------------------------------
# Packed (sparse-mask) fast path
# --------------------------------------------------------------------------

PACK_MAX = 1024    # per-layer packed contraction cap (SBUF/PSUM budget)


def _rup(n, m=P):
    return max(m, (n + m - 1) // m * m)


def _kpad(n):
    """Contraction-dim padding: under fp8, round K up to an EVEN number of
    128-subtiles so every matmul runs in DoubleRow mode (an all-DR K of
    2j subtiles costs the same as j single-subtile matmuls — the zero pad
    planes are free)."""
    if DTYPE == "fp8":
        return _rup(n, 2 * P)
    return n


def plan_packed(m1, m2, m3):
    """Assign output rows to cores (greedy set-union balancing: rows
    sharing h2 features cluster together, minimizing each core's used-
    feature count), then walk the masks backwards per core. Returns
    (sizes (K1, F1, F2), per-core (S1, S2, S3, rows)) or None if any
    packed dim exceeds PACK_MAX."""
    m1 = np.asarray(m1)
    m2 = np.asarray(m2)
    m3 = np.asarray(m3)
    fs3 = DIMS[3] // NCORES

    # Constant-feature fold (mask-only liveness, recursive): h1 features
    # with empty m1 rows are batch-constant tanh(b1) and fold into an
    # adjusted b2; h2 features whose m2 support is all-constant are then
    # batch-constant tanh(b2_adj) and fold into the output bias. Only
    # "live" features and rows touching them reach the device.
    live1 = m1.any(axis=1)
    live2 = (m2 & live1[None, :]).any(axis=1)
    cols_of = [np.flatnonzero(m3[r] & live2) for r in range(DIMS[3])]
    nz = [r for r in range(DIMS[3]) if len(cols_of[r])]
    zr = [r for r in range(DIMS[3]) if not len(cols_of[r])]
    nz.sort(key=lambda r: -len(cols_of[r]))
    # Joint objective: primarily balance the induced h1-feature unions
    # (|S2| drives the ScalarE tanh chain AND layer 2's contraction),
    # secondarily the h2 unions, with soft caps one pad-class down.
    rowcols2 = {}
    for r in nz:
        for c in cols_of[r]:
            if c not in rowcols2:
                rowcols2[c] = np.flatnonzero(m2[c] & live1)
    CAP3, CAP2 = 3 * P - 1, 2 * P - 1
    mem3 = np.zeros((NCORES, DIMS[2]), bool)
    mem2 = np.zeros((NCORES, DIMS[1]), bool)
    n3 = [0] * NCORES
    n2 = [0] * NCORES
    cnt = [0] * NCORES
    assign = [[] for _ in range(NCORES)]
    for r in nz:
        cs = cols_of[r]
        best, bestcost = None, None
        for k in range(NCORES):
            if cnt[k] >= fs3:
                continue
            new3 = [c for c in cs if not mem3[k, c]]
            new2 = sum(int((~mem2[k, rowcols2[c]]).sum()) for c in new3)
            pen = (10000 if n3[k] + len(new3) > CAP3 else 0) + \
                  (10000 if n2[k] + new2 > CAP2 else 0)
            cost = (pen + new2 + 0.3 * len(new3), n2[k], cnt[k])
            if bestcost is None or cost < bestcost:
                best, bestcost = k, cost
        k = best
        for c in cs:
            if not mem3[k, c]:
                mem2[k, rowcols2[c]] = True
        mem3[k, cs] = True
        n3[k] = int(mem3[k].sum())
        n2[k] = int(mem2[k].sum())
        cnt[k] += 1
        assign[k].append(r)
    # Rows whose m3 row is all-zero produce exactly b3 (and, in prod8
    # mode, exactly 0 on device) — they never touch the device. Each core
    # computes only its nonzero rows, padded to the shared f3 size.
    idxs = []
    k1 = f1 = f2 = f3 = 0
    for k in range(NCORES):
        rows = np.array(sorted(assign[k]), dtype=np.int64)
        S3 = np.flatnonzero(m3[rows].any(axis=0) & live2) if len(rows) \
            else np.zeros(0, np.int64)
        S2 = np.flatnonzero(m2[S3].any(axis=0) & live1)
        S1 = np.flatnonzero(m1[S2].any(axis=0))
        if len(S3) > PACK_MAX or len(S2) > PACK_MAX or len(S1) > PACK_MAX:
            return None
        idxs.append((S1, S2, S3, rows))
        k1, f1 = max(k1, len(S1)), max(f1, len(S2))
        f2, f3 = max(f2, len(S3)), max(f3, len(rows))
    return (_rup(k1), _rup(f1), _rup(f2), min(_rup(f3), fs3)), idxs


def _b2_adjusted(b1, b2, W2, m1, m2):
    """b2 with the constant h1 features' contributions folded in:
    b2_adj[c] = b2[c] + sum_{i: m1 row i empty} W2m[c,i] * tanh(b1[i])."""
    live1 = np.asarray(m1).any(axis=1)
    th1 = np.tanh(np.asarray(b1, np.float32))
    r2, c2 = np.nonzero(np.asarray(m2))
    sel = ~live1[c2]
    b2a = np.asarray(b2, np.float32).copy()
    np.add.at(b2a, r2[sel],
              np.asarray(W2, np.float32)[r2[sel], c2[sel]] * th1[c2[sel]])
    return b2a


def _build_packed(k1, f1, f2, f3=None, rep=None):
    """Single-core-local packed MLP: [k1]->[f1]->[f2]->[512], B=2048.
    Same NEFF on all 8 cores; per-core inputs differ. No collectives.
    rep (env BASS_MLP_REP, default 1) unrolls the compute pipeline for
    device-time measurement via chain-marginal differencing.

    All weights/activations stay SBUF-resident. Work is tiled per single
    512-batch block (one PSUM bank): finer granularity starts the serial
    ScalarE tanh chain ~1.5us earlier and overlaps the ~0.9us DMA-
    completion semaphore latencies across blocks. The fp8 DoubleRow zero
    pad planes of x/h are built on device (memsets off the critical path)
    so the x DMA moves only real bytes. Layer-3 evictions alternate
    DVE/ScalarE; the last block splits into halves across both engines."""
    import concourse.tile as tile
    from concourse import bacc, mybir
    from concourse.bass import DynSlice

    cdt = {
        "fp8": mybir.dt.float8e4,
        "fp16": mybir.dt.float16,
        "bf16": mybir.dt.bfloat16,
        "fp32r": mybir.dt.float32r,
        "fp32": mybir.dt.float32,
    }[DTYPE]
    odt = {"fp32": mybir.dt.float32,
           "prod8": mybir.dt.float8e4}.get(OUT_DT, mybir.dt.float16)
    sw, sx = _scales()
    dscale = [1.0 / (sw * sx), 1.0 / sw, 1.0 / sw]   # PSUM descale per layer
    prod8 = OUT_DT == "prod8"
    if prod8:
        dscale[2] *= OSCALE
    use_dr = DTYPE == "fp8"

    if f3 is None:
        f3 = DIMS[3] // NCORES                 # output rows per core
    KS = [_kpad(k1), _kpad(f1), _kpad(f2)]     # contraction per layer (padded)
    FS = [f1, f2, f3]                          # output features per layer
    BOFF = [0, f1 // P, (f1 + f2) // P]        # bias column offsets

    nc = bacc.Bacc(None, target_bir_lowering=False, debug=False,
                   num_devices=NCORES)

    # xp carries only the REAL k1 rows; the fp8 DoubleRow zero pad planes
    # are memset on device, halving the x DMA (the L1 critical path).
    # All three weight matrices AND the fp32 biases (shipped as raw bytes,
    # fp32-bitcast on device) ride ONE fused DRAM tensor/DMA: the HWDGE
    # descriptor-gen and the DMA transfer engine are single shared
    # resources, so every extra input DMA adds ~2us of serial latency
    # (desc-gen + launch + completion-sem) to the input phase no matter
    # which queue it rides.
    WOFF = []   # per-layer column offset into the fused [P, wcols] tile
    wcols = 0
    for li in range(3):
        WOFF.append(wcols)
        wcols += (KS[li] // P) * FS[li]
    BOFFB = wcols                        # bias bytes offset
    nbias = (f1 + f2 + f3) // P          # fp32 bias columns per partition
    wcols += 4 * nbias
    XOFF = wcols                         # x columns offset ([ko, B] flat)
    wcols += (k1 // P) * B
    wall = nc.dram_tensor("wall", [P, wcols], cdt, kind="ExternalInput")
    out = nc.dram_tensor("out", [f3, B], odt, kind="ExternalOutput")

    with tile.TileContext(nc) as tc:
        with tc.tile_pool(name="per", bufs=1) as per, \
             tc.tile_pool(name="op", bufs=8) as opool, \
             tc.tile_pool(name="ps", bufs=4, space="PSUM") as pspool:

            # ---- persistent SBUF residents ----
            # x/h tiles are sized to their layer's padded contraction; pad
            # planes beyond the real features are zeroed once below, off
            # the critical path (Pool for x, DVE for h — both idle early).
            tcols = XOFF + (KS[0] // P) * B   # + device-side DR pad planes
            wt_all = per.tile([P, tcols], cdt, tag="wall", name="wt_all")
            xt = wt_all[:, XOFF:tcols].rearrange("p (ko n) -> p ko n",
                                                 ko=KS[0] // P)
            wt = [wt_all[:, WOFF[li]:WOFF[li] + (KS[li] // P) * FS[li]]
                  .rearrange("p (ko f) -> p ko f", ko=KS[li] // P)
                  for li in range(3)]
            h = [per.tile([P, KS[li + 1] // P, B], cdt, tag=f"h{li}",
                          name=f"ht{li}") for li in range(2)]
            bt = wt_all[:, BOFFB:BOFFB + 4 * nbias].bitcast(mybir.dt.float32)
            # Input DMAs: all on the sync queue in first-use order — the
            # fused weights, then x per 512-batch block with the bias
            # tucked in after the first block (the shared desc-gen and
            # transfer engines serialize everything anyway, so order is
            # the only lever). DoubleRow pad-plane memsets: x and h2 on
            # Pool, h1 on DVE — each finishes well before its first reader.
            # ONE input DMA carries weights, bias bytes AND x: with the
            # shared desc-gen/launch/completion-sem latencies (~2.2us per
            # DMA chain), a single blob beats chunked x arrivals — the
            # whole input lands before the first chunked x0 would have
            # cleared its own semaphore, and the tanh chain then runs
            # back-to-back with no x-pacing stalls.
            nc.sync.dma_start(wt_all[:, 0:wcols], wall.ap())

            # dummy 1-element tanh: pulls the ACT function-table load into
            # the DMA head instead of delaying the first real eviction
            warm = per.tile([1, 1], mybir.dt.float32, tag="warm", name="warm")
            nc.gpsimd.memset(warm[:], 0.0)
            nc.scalar.activation(warm[:], warm[:],
                                 mybir.ActivationFunctionType.Tanh)


            if KS[0] > k1:
                nc.gpsimd.memset(wt_all[:, XOFF + (k1 // P) * B:tcols], 0.0)
            if KS[1] > FS[0]:
                nc.vector.memset(
                    h[0][:, slice(FS[0] // P, KS[1] // P), :], 0.0)
            if KS[2] > FS[1]:
                nc.gpsimd.memset(
                    h[1][:, slice(FS[1] // P, KS[2] // P), :], 0.0)

            # out-DMA queues: all on sync (idle after the input loads, and
            # HWDGE desc-gen at 625ns beats gpsimd's 1038ns SWDGE). Never
            # scalar — that queue shares the ACT sequencer and a waiting
            # dma_start would head-of-line-block the eviction dispatches.
            oqs = [nc.sync] * 7
            if rep is None:
                rep = int(os.environ.get("BASS_MLP_REP", "1"))
            for _r in range(rep):
                _layers(nc, tc, mybir, DynSlice, opool, pspool, oqs,
                        KS, FS, BOFF, xt, wt, h, bt, out,
                        use_dr, dscale, odt, _r, prod8)

    nc.compile()
    return nc


def _layers(nc, tc, mybir, DynSlice, opool, pspool, oqs,
            KS, FS, BOFF, xt, wt, h, bt, out, use_dr, dscale, odt, _r,
            prod8):
            # Batch granularity per layer: L1 evicts per single 512-block
            # (the ACT tanh chain starts right after the first block's
            # matmul), L2 per 1024-pair (fewer ACT ops once the chain is
            # saturated), L3 per 512-block on DVE+Pool in parallel (off
            # the ACT chain), flushed as two [128,1024] DMAs — out-DMA
            # desc-gens serialize on the shared HWDGE, so fewer is faster.
            def mms(li, pdst, wsl, bstart):
                KO = KS[li] // P
                # L1 matmuls start from a cold (p-state-ramped) PE after
                # each x-chunk wait: issue as two back-to-back halves so
                # only the first rides the slow ramp
                nsub = 2 if li == 0 else 1
                sw = FD // nsub
                ko = 0
                while ko < KO:
                    dr = use_dr and ko + 1 < KO
                    step = 2 if dr else 1
                    pm = (mybir.MatmulPerfMode.DoubleRow if dr else None)
                    for s in range(nsub):
                        src = (xt if li == 0 else h[li - 1])[
                            :, slice(ko, ko + step),
                            DynSlice(bstart + s * sw, sw)]
                        nc.tensor.matmul(
                            pdst[:, DynSlice(s * sw, sw)] if nsub > 1
                            else pdst,
                            wt[li][:, slice(ko, ko + step), wsl], src,
                            perf_mode=pm,
                            start=(ko == 0), stop=(ko + step >= KO))
                    ko += step

            # L1 evicts per single 512-block (the ACT tanh chain starts
            # right after the first block's matmul and stays x-paced),
            # L2 evicts 1024-pairs (fewer ACT ops once the chain runs).
            GRPS = [tuple((b * FD, FD) for b in range(NB)),
                    ((0, 2 * FD), (2 * FD, 2 * FD))]
            for li in range(2):
                for g0, bw in GRPS[li]:
                    gsl = DynSlice(g0, bw)
                    for f in range(FS[li] // P):
                        wsl = DynSlice(f * P, P)
                        # uniform 2-bank slots (one tag) so the pool fits
                        # PSUM exactly; 512-wide users take the low half
                        pfull = pspool.tile([P, 2 * FD], mybir.dt.float32,
                                            tag="ps",
                                            name=f"ps{_r}_{li}_{f}_{g0}")
                        for bb in range(bw // FD):
                            mms(li, pfull[:, DynSlice(bb * FD, FD)], wsl,
                                g0 + bb * FD)
                        nc.scalar.activation(
                            h[li][:, f, gsl], pfull[:, 0:bw],
                            mybir.ActivationFunctionType.Tanh,
                            bias=bt[:, DynSlice(BOFF[li] + f, 1)],
                            scale=dscale[li])

            # L3: per-512-block psum tiles (a shared tile would WAR-stall
            # the next block's matmul behind this block's eviction), then
            # evictions alternating DVE/ACT (GPSIMD cannot read PSUM; ACT
            # is free once the tanh chain ends), flushed as two
            # [128,1024] DMAs — out-DMA desc-gens serialize on the shared
            # HWDGE, so fewer is faster.
            engs = [nc.vector, nc.scalar, nc.scalar, nc.vector]
            for f in range(FS[2] // P):
                wsl = DynSlice(f * P, P)
                bias = bt[:, DynSlice(BOFF[2] + f, 1)]
                ots = [opool.tile([P, 2 * FD], odt, tag="prod",
                                  name=f"o{_r}_{f}_{g}") for g in range(2)]
                for b in range(NB):
                    pfull = pspool.tile([P, 2 * FD], mybir.dt.float32,
                                        tag="ps", name=f"ps{_r}_2_{f}_{b}")
                    psl = pfull[:, 0:FD]
                    mms(2, psl, wsl, b * FD)
                    osl = ots[b // 2][:, DynSlice((b % 2) * FD, FD)]
                    eng = engs[b]
                    if eng is nc.scalar:
                        nc.scalar.activation(
                            osl, psl,
                            mybir.ActivationFunctionType.Identity,
                            bias=0.0 if prod8 else bias, scale=dscale[2])
                    elif prod8:
                        eng.tensor_scalar_mul(osl, psl, dscale[2])
                    else:
                        eng.tensor_scalar(osl, psl, dscale[2], bias,
                                          mybir.AluOpType.mult,
                                          mybir.AluOpType.add)
                    if b % 2 == 1:
                        nc.sync.dma_start(
                            out.ap()[wsl, DynSlice((b - 1) * FD, 2 * FD)],
                            ots[b // 2][:])


def make_in_maps_packed(x, W1, b1, m1, W2, b2, m2, W3, b3, m3, sizes, idxs):
    """Gather per-core packed (and for fp8, pre-scaled) submatrices plus
    the concatenated fp32 bias vector."""
    k1, f1, f2, f3 = sizes
    npdt = _np_cdt()
    sw, sx = _scales()
    x, W1, b1, m1, W2, b2, m2, W3, b3, m3 = (
        np.asarray(a) for a in (x, W1, b1, m1, W2, b2, m2, W3, b3, m3))
    b2a = _b2_adjusted(b1, b2, W2, m1, m2)
    in_maps = []
    for k in range(NCORES):
        S1, S2, S3, rows = idxs[k]
        m = {}
        xk = np.zeros((k1, B), npdt)
        xk[:len(S1)] = (x[:, S1].T * sx).astype(npdt) if sx != 1.0 \
            else x[:, S1].T

        w1 = np.zeros((_kpad(k1), f1), npdt)
        w1[:len(S1), :len(S2)] = (
            (W1[np.ix_(S2, S1)] * m1[np.ix_(S2, S1)]).T * sw)
        w2 = np.zeros((_kpad(f1), f2), npdt)
        w2[:len(S2), :len(S3)] = (
            (W2[np.ix_(S3, S2)] * m2[np.ix_(S3, S2)]).T * sw)
        w3 = np.zeros((_kpad(f2), f3), npdt)
        w3[:len(S3), :len(rows)] = (
            (W3[np.ix_(rows, S3)] * m3[np.ix_(rows, S3)]).T * sw)
        bv = np.zeros(f1 + f2 + f3, np.float32)
        bv[:len(S2)] = b1[S2]
        bv[f1:f1 + len(S3)] = b2a[S3]
        bv[f1 + f2:f1 + f2 + len(rows)] = b3[rows]
        # fused weight tensor: each w [(ko p), f] -> [p, ko*f], the three
        # concatenated along columns (matches the device's WOFF layout),
        # then the fp32 bias bytes (bitcast back to fp32 on device)
        bbytes = np.ascontiguousarray(
            bv.reshape(-1, P).T).view(np.uint8).view(npdt)
        m["wall"] = np.concatenate(
            [w.reshape(-1, P, w.shape[1]).transpose(1, 0, 2).reshape(P, -1)
             for w in (w1, w2, w3)] + [bbytes] +
            [xk.reshape(-1, P, B).transpose(1, 0, 2).reshape(P, -1)], axis=1)
        in_maps.append(m)
    return in_maps


# --------------------------------------------------------------------------
# Dense fallback (Megatron column-parallel, AllGather after layers 1/2)
# --------------------------------------------------------------------------

def _build(l1k=DIMS[0]):
    """Build + schedule the SPMD Bass program (same NEFF on all 8 cores).

    l1k: layer-1 contraction size. DIMS[0] for the dense path; a smaller
    multiple of 512 when the host packs only the K-rows that survive m1
    (per-core), padding with zeros.
    """
    import concourse.tile as tile
    from concourse import bacc, mybir
    from concourse.bass import DynSlice

    cdt = {
        "fp16": mybir.dt.float16,
        "bf16": mybir.dt.bfloat16,
        "fp32r": mybir.dt.float32r,  # rounded fp32; np side is float32
        "fp32": mybir.dt.float32,
    }[_dense_dtype()]
    esz = mybir.dt.size(cdt)

    # Per-layer output-feature shard sizes and weight-panel widths.
    FS = [DIMS[1] // NCORES, DIMS[2] // NCORES, DIMS[3] // NCORES]  # 1024,1024,512
    KS = [l1k, DIMS[1], DIMS[2]]
    if esz == 2:
        # Uniform 64KB/partition weight-panel slots so wpool can double-buffer:
        # the next panel's DMA+mask overlaps the current panel's matmuls.
        FBLK = [1024, 512, 512]
        mck, ibufs, wbufs = MCK, 6, 2
    else:
        FBLK = [1024, 512, 512]      # L2 split into two panels (SBUF)
        mck, ibufs, wbufs = 2, 4, 1

    nc = bacc.Bacc(None, target_bir_lowering=False, debug=False, num_devices=NCORES)

    xT = nc.dram_tensor("xT", [KS[0], B], cdt, kind="ExternalInput")
    wts, mts, bs = [], [], []
    for li in range(3):
        wts.append(nc.dram_tensor(f"w{li + 1}t", [KS[li], FS[li]], cdt,
                                  kind="ExternalInput"))
        mts.append(nc.dram_tensor(f"m{li + 1}t", [KS[li], FS[li]], cdt,
                                  kind="ExternalInput"))
        bs.append(nc.dram_tensor(f"b{li + 1}", [FS[li]], mybir.dt.float32,
                                 kind="ExternalInput"))
    out = nc.dram_tensor("out", [FS[2], B], mybir.dt.float32,
                         kind="ExternalOutput")

    with tile.TileContext(nc) as tc:
        with tc.tile_pool(name="wp", bufs=wbufs) as wpool, \
             tc.tile_pool(name="inp", bufs=ibufs) as ipool, \
             tc.tile_pool(name="mp", bufs=2) as mpool, \
             tc.tile_pool(name="op", bufs=6) as opool, \
             tc.tile_pool(name="bp", bufs=3) as bpool, \
             tc.tile_pool(name="ps", bufs=8, space="PSUM") as pspool, \
             tc.tile_pool(name="dram", bufs=1, space="DRAM") as dram:

            # Per-(layer, b-block) activation tensors so each AllGather covers
            # one 512-batch block and pipelines behind compute.
            h_loc = [[dram.tile([FS[li], FD], cdt, name=f"h{li + 1}_loc{b}")
                      for b in range(NB)] for li in range(2)]
            h_full = [[dram.tile([DIMS[li + 1], FD], cdt, addr_space="Shared",
                                 name=f"h{li + 1}_full{b}")
                       for b in range(NB)] for li in range(2)]

            def layer(li, tanh):
                K, F = KS[li], FS[li]
                KO = K // P
                wt_r = wts[li].ap().rearrange("(ko p) f -> p ko f", p=P)
                mt_r = mts[li].ap().rearrange("(ko p) f -> p ko f", p=P)
                if li == 0:
                    xr = xT.ap().rearrange("(ko p) n -> p ko n", p=P)
                    in_rs = [xr[:, :, DynSlice(b * FD, FD)] for b in range(NB)]
                else:
                    in_rs = [h_full[li - 1][b][:].rearrange(
                        "(ko p) n -> p ko n", p=P) for b in range(NB)]

                btile = bpool.tile([P, F // P], mybir.dt.float32, tag="bias",
                                   name=f"bias{li}")
                nc.sync.dma_start(btile[:], bs[li].ap().rearrange(
                    "(o p) -> p o", p=P))

                fblk = FBLK[li]
                for f0 in range(0, F, fblk):
                    # --- load + mask one weight panel [P, KO, fblk] ---
                    wp = wpool.tile([P, KO, fblk], cdt, tag="wpanel",
                                    name=f"wp{li}_{f0}")
                    # weight/mask loads go on gpsimd/vector DMA queues so the
                    # input-strip stream on the sync queue is never stuck
                    # behind a 16MB panel load
                    for c0 in range(0, KO, mck):
                        csl = slice(c0, c0 + mck)
                        fsl = DynSlice(f0, fblk)
                        nc.gpsimd.dma_start(wp[:, csl, :], wt_r[:, csl, fsl])
                        mtile = mpool.tile([P, mck, fblk], cdt, tag="mchunk",
                                           name=f"m{li}_{f0}_{c0}")
                        nc.gpsimd.dma_start(mtile[:], mt_r[:, csl, fsl])
                        nc.vector.tensor_tensor(wp[:, csl, :], wp[:, csl, :],
                                                mtile[:], mybir.AluOpType.mult)

                    nf = fblk // P
                    for b in range(NB):
                        psums = [pspool.tile([P, FD], mybir.dt.float32,
                                             tag="ps", name=f"ps{li}_{f0}_{b}_{f}")
                                 for f in range(nf)]
                        for c0 in range(0, KO, ICK):
                            it = ipool.tile([P, ICK, FD], cdt, tag="instrip",
                                            name=f"in{li}_{f0}_{b}_{c0}")
                            nc.sync.dma_start(
                                it[:], in_rs[b][:, slice(c0, c0 + ICK), :])
                            for f in range(nf):
                                for ks in range(ICK):
                                    ko = c0 + ks
                                    nc.tensor.matmul(
                                        psums[f][:],
                                        wp[:, ko, DynSlice(f * P, P)],
                                        it[:, ks, :],
                                        start=(ko == 0), stop=(ko == KO - 1))
                        for f in range(nf):
                            fg = f0 + f * P   # feature row offset in shard
                            odt = cdt if li < 2 else mybir.dt.float32
                            ot = opool.tile([P, FD], odt, tag="prod",
                                            name=f"o{li}_{f0}_{b}_{f}")
                            func = (mybir.ActivationFunctionType.Tanh if tanh
                                    else mybir.ActivationFunctionType.Identity)
                            nc.scalar.activation(
                                ot[:], psums[f][:], func,
                                bias=btile[:, DynSlice((f0 // P) + f, 1)])
                            if li < 2:
                                nc.sync.dma_start(
                                    h_loc[li][b][DynSlice(fg, P), :], ot[:])
                            else:
                                nc.sync.dma_start(
                                    out.ap()[DynSlice(fg, P),
                                             DynSlice(b * FD, FD)], ot[:])
                        # fire this b-block's AllGather as soon as the last
                        # panel has written it
                        if li < 2 and f0 == F - fblk:
                            nc.gpsimd.collective_compute(
                                "AllGather",
                                mybir.AluOpType.bypass,
                                replica_groups=[list(range(NCORES))],
                                ins=[h_loc[li][b].opt()],
                                outs=[h_full[li][b].opt()],
                            )

            layer(0, tanh=True)
            layer(1, tanh=True)
            layer(2, tanh=False)

    nc.compile()
    return nc


PACK_K = 512   # packed layer-1 contraction size (dense-path fast variant)


def get_nc(l1k=DIMS[0]):
    key = ("dense", l1k)
    if key not in _cache:
        _cache[key] = _build(l1k)
    return _cache[key]


def get_nc_packed(sizes):
    key = ("packed", sizes, os.environ.get("BASS_MLP_REP", "1"))
    if key not in _cache:
        _cache[key] = _build_packed(*sizes)
    return _cache[key]


def plan_l1k(m1):
    """If m1 is sparse enough that every core's shard of (W1*m1).T touches at
    most PACK_K input dims, return (PACK_K, per-core used-row indices); else
    the dense plan."""
    m1 = np.asarray(m1)
    fs = DIMS[1] // NCORES
    idxs = []
    for k in range(NCORES):
        idx = np.flatnonzero(m1[k * fs:(k + 1) * fs].any(axis=0))
        if len(idx) > PACK_K:
            return DIMS[0], None
        idxs.append(idx)
    return PACK_K, idxs


def make_in_maps(x, W1, b1, m1, W2, b2, m2, W3, b3, m3, idxs=None):
    """Host-side sharding: transpose to [K, F] layouts, cast, slice shards.
    With idxs, layer-1 operands are gathered to the PACK_K used K-rows."""
    x, W1, b1, m1, W2, b2, m2, W3, b3, m3 = (
        np.asarray(a) for a in (x, W1, b1, m1, W2, b2, m2, W3, b3, m3))
    npdt = _np_dense_dt()
    xT = np.ascontiguousarray(x.T).astype(npdt, copy=False)
    Ws = [W1, W2, W3]
    Ms = [m1, m2, m3]
    Bs = [b1, b2, b3]
    in_maps = []
    for k in range(NCORES):
        m = {}
        for li in range(3):
            F = DIMS[li + 1]
            fs = F // NCORES
            sl = slice(k * fs, (k + 1) * fs)
            wt = Ws[li][sl].T
            mt = Ms[li][sl].T
            if li == 0:
                if idxs is None:
                    m["xT"] = xT
                else:
                    idx = idxs[k]
                    xk = np.zeros((PACK_K, B), npdt)
                    xk[:len(idx)] = xT[idx]
                    m["xT"] = xk
                    wk = np.zeros((PACK_K, fs), npdt)
                    wk[:len(idx)] = wt[idx].astype(npdt)
                    mk = np.zeros((PACK_K, fs), npdt)
                    mk[:len(idx)] = mt[idx].astype(npdt)
                    m["w1t"], m["m1t"] = wk, mk
            if f"w{li + 1}t" not in m:
                m[f"w{li + 1}t"] = np.ascontiguousarray(wt).astype(
                    npdt, copy=False)
                m[f"m{li + 1}t"] = np.ascontiguousarray(mt).astype(npdt)
            m[f"b{li + 1}"] = np.ascontiguousarray(Bs[li][sl]).astype(
                np.float32, copy=False)
        in_maps.append(m)
    return in_maps


def prepare(x, W1, b1, m1, W2, b2, m2, W3, b3, m3):
    """Plan, build (cached), and shard: returns (nc, in_maps, rowperm).
    rowperm is the output-row order produced by concatenating the per-core
    results (None for the dense fallback's natural order)."""
    plan = plan_packed(m1, m2, m3)
    if plan is not None:
        sizes, idxs = plan
        nc = get_nc_packed(sizes)
        in_maps = make_in_maps_packed(x, W1, b1, m1, W2, b2, m2, W3, b3, m3,
                                      sizes, idxs)
        rowperm = (sizes[3], [rows for _, _, _, rows in idxs])
    else:
        l1k, idxs = plan_l1k(m1)
        nc = get_nc(l1k)
        in_maps = make_in_maps(x, W1, b1, m1, W2, b2, m2, W3, b3, m3,
                               idxs=idxs)
        rowperm = None
    return nc, in_maps, rowperm


def kernel(x, W1, b1, m1, W2, b2, m2, W3, b3, m3):
    from concourse.bass_utils import run_bass_kernel_spmd

    nc, in_maps, rowperm = prepare(x, W1, b1, m1, W2, b2, m2, W3, b3, m3)
    res = run_bass_kernel_spmd(nc, in_maps, core_ids=list(range(NCORES)))
    if rowperm is not None:
        # scatter each core's real rows; rows the device never computed
        # are pure (adjusted) bias. Constant h2 features (empty m2 rows,
        # h2 = tanh(b2)) were excluded on device — fold their output
        # contribution in here.
        f3, row_lists = rowperm
        b3f = np.asarray(b3, np.float32)
        m1a = np.asarray(m1)
        m2a = np.asarray(m2)
        m3a = np.asarray(m3)
        W3f = np.asarray(W3, np.float32)
        b2a = _b2_adjusted(b1, b2, W2, m1, m2)
        live2 = (m2a & m1a.any(axis=1)[None, :]).any(axis=1)
        r3, c3 = np.nonzero(m3a)
        cm = ~live2[c3]
        fold = np.zeros(DIMS[3], np.float32)
        np.add.at(fold, r3[cm],
                  W3f[r3[cm], c3[cm]] * np.tanh(b2a[c3[cm]]))
        full = np.zeros((DIMS[3], B), np.float32)
        for k in range(NCORES):
            rk = row_lists[k]
            full[rk] = np.asarray(res.results[k]["out"][:len(rk)],
                                  np.float32)
        if OUT_DT == "prod8":
            # device emitted OSCALE * (h2 @ W3masked); add biases here
            full = full / OSCALE + (b3f + fold)[:, None]
        else:
            covered = np.zeros(DIMS[3], bool)
            for rk in row_lists:
                covered[rk] = True
            full[~covered] = b3f[~covered, None]
            full += fold[:, None]
        return np.ascontiguousarray(full.T)
    outT = np.concatenate([res.results[k]["out"] for k in range(NCORES)],
                          axis=0).astype(np.float32, copy=False)
    return np.ascontiguousarray(outT.T)



# revision 44
# speedup vs baseline: 1.7689x; 1.7689x over previous
"""Masked 3-layer MLP (tanh) on 8 Trainium2 NeuronCores.

Reference computation (B=2048, dims 4096->8192->8192->4096, fp32):
    h1 = tanh(x @ (W1*m1).T + b1)
    h2 = tanh(h1 @ (W2*m2).T + b2)
    out =      h2 @ (W3*m3).T + b3

The masks are p=1e-4 Bernoulli, so the effective network is tiny. Fast
path ("packed"): output rows are assigned to cores by a greedy set-union
clustering (rows sharing h2 features land on the same core, which hits
the theoretical minimum of ~344 used h2 features per core). Walking the
masks backwards from each core's row set: needed h2 features S3_k, then
h1 features S2_k = nonzero m2 columns over S3_k, then x dims S1_k. The
host gathers the masked weight submatrices over exactly those index sets
(zero-padded to shared multiples of 128), and each core runs a fully
LOCAL dense 3-layer MLP with contractions ~128->256->384 instead of
4096->8192->8192. No collectives, no DRAM intermediates: weights, the
x-pack and both hidden activations stay SBUF-resident; only the final
[512, B] shard is written out (fp8 product-only by default — the output
is ~96% bias by magnitude, so the host adds the exact fp32 bias).

Compute is in transposed orientation [features, batch]: output features
land on PSUM partitions, so the per-partition bias + descale + tanh fuse
into the PSUM eviction. Default compute dtype is fp8 e4m3 with DoubleRow
matmuls (2 K-subtiles per instruction at 2x rate); weights are host-
scaled by 128 (above e4m3's min-normal), x by 16, both undone exactly by
the eviction's power-of-two `scale`; biases stay exact fp32, which keeps
rel err ~1e-3.

The kernel is DMA-latency- and ScalarE-chain-bound, so the schedule is
built around two shared single resources (HWDGE descriptor-gen, ~625ns
per DMA, and the transfer engine) plus the serial ACT tanh chain:
  - ONE fused input tensor/DMA carries w1|w2|w3, the fp32 biases as raw
    bytes (fp32-bitcast on device), AND the x pack: a single blob beats
    chunked x arrivals because every extra DMA pays desc-gen + launch +
    completion-sem (~2.2us of serial latency), and it leaves just one
    descriptor ring for the runtime to re-arm per execution. x/h
    DoubleRow zero pad planes are memset on device (Pool/DVE, off the
    critical path) instead of DMA'd.
  - L1 evicts per 512-block (the tanh chain starts one matmul after the
    input lands and runs back-to-back); L1 matmuls issue as two 256
    halves so only the first rides the PE p-state ramp. L2 evicts
    1024-pairs (fewer ACT ops once saturated).
  - L3 runs per-512-block psum tiles (a shared tile would WAR-stall the
    next matmul behind the previous eviction), evictions alternating
    DVE/ACT (GPSIMD cannot read PSUM), flushed as two [128,1024] DMAs.

Fallback (masks not sparse enough to pack): the previous Megatron-style
column-parallel dense fp16 kernel with on-chip AllGathers after layers
1/2.
"""

import os
import sys

import numpy as np

for _p in ("/opt/trn_rl_repo", os.path.expanduser("~/.axon_site/_ro/trn_rl_repo")):
    if os.path.isdir(_p) and _p not in sys.path:
        sys.path.append(_p)

B = 2048
DIMS = [4096, 8192, 8192, 4096]
NCORES = 8
P = 128
FD = 512           # matmul moving free dim == one PSUM bank of fp32
NB = B // FD       # batch blocks
ICK = 4            # K-subtiles (x128 rows) per streamed input chunk
MCK = 4            # K-subtiles per weight/mask load+mask chunk

# Compute dtype: fp8 | fp16 | bf16 | fp32r | fp32
DTYPE = os.environ.get("BASS_MLP_DTYPE", "fp8")
# Output mode: "fp32" | "cdt" (fp16) | "prod8" (fp8 product-only: the
# device emits h2@W3 scaled by OSCALE without bias — the output is ~96%
# bias by magnitude, so the host adds exact fp32 b3 and the fp8 product
# quantization is negligible; out DMA shrinks to 1MB/core)
OUT_DT = os.environ.get("BASS_MLP_OUT_DT", "prod8")
OSCALE = 131072.0   # 2**17; device product absmax ~4e-4 -> ~52 in e4m3
# fp8 pre-scales: weights sit near e4m3's min-normal (0.0156), so scale
# them up into the normal range; x gets a mild scale for its small tail.
# The product scale is undone exactly (power of two) by the activation's
# `scale` parameter at PSUM-eviction time. Biases stay exact fp32.
SCALE_W = 128.0
SCALE_X = 16.0

_cache = {}


def _np_cdt():
    if DTYPE in ("bf16", "fp8"):
        import ml_dtypes

        return {"bf16": ml_dtypes.bfloat16,
                "fp8": ml_dtypes.float8_e4m3}[DTYPE]
    return {"fp16": np.float16, "fp32r": np.float32, "fp32": np.float32}[DTYPE]


def _scales():
    if DTYPE == "fp8":
        return SCALE_W, SCALE_X
    return 1.0, 1.0


# The dense fallback has K up to 8192 and no per-layer rescaling; run it in
# fp16 when the packed path's fp8 dtype is selected.
def _dense_dtype():
    return "fp16" if DTYPE == "fp8" else DTYPE


def _np_dense_dt():
    if _dense_dtype() == "bf16":
        import ml_dtypes

        return ml_dtypes.bfloat16
    return {"fp16": np.float16, "fp32r": np.float32,
            "fp32": np.float32}[_dense_dtype()]


# --------------------------------------------------------------------------
# Packed (sparse-mask) fast path
# --------------------------------------------------------------------------

PACK_MAX = 1024    # per-layer packed contraction cap (SBUF/PSUM budget)


def _rup(n, m=P):
    return max(m, (n + m - 1) // m * m)


def _kpad(n):
    """Contraction-dim padding: under fp8, round K up to an EVEN number of
    128-subtiles so every matmul runs in DoubleRow mode (an all-DR K of
    2j subtiles costs the same as j single-subtile matmuls — the zero pad
    planes are free)."""
    if DTYPE == "fp8":
        return _rup(n, 2 * P)
    return n


def plan_packed(m1, m2, m3):
    """Assign output rows to cores (greedy set-union balancing: rows
    sharing h2 features cluster together, minimizing each core's used-
    feature count), then walk the masks backwards per core. Returns
    (sizes (K1, F1, F2), per-core (S1, S2, S3, rows)) or None if any
    packed dim exceeds PACK_MAX."""
    m1 = np.asarray(m1)
    m2 = np.asarray(m2)
    m3 = np.asarray(m3)
    fs3 = DIMS[3] // NCORES

    # Constant-feature fold (mask-only liveness, recursive): h1 features
    # with empty m1 rows are batch-constant tanh(b1) and fold into an
    # adjusted b2; h2 features whose m2 support is all-constant are then
    # batch-constant tanh(b2_adj) and fold into the output bias. Only
    # "live" features and rows touching them reach the device.
    live1 = m1.any(axis=1)
    live2 = (m2 & live1[None, :]).any(axis=1)
    cols_of = [np.flatnonzero(m3[r] & live2) for r in range(DIMS[3])]
    nz = [r for r in range(DIMS[3]) if len(cols_of[r])]
    zr = [r for r in range(DIMS[3]) if not len(cols_of[r])]
    nz.sort(key=lambda r: -len(cols_of[r]))
    # Joint objective: primarily balance the induced h1-feature unions
    # (|S2| drives the ScalarE tanh chain AND layer 2's contraction),
    # secondarily the h2 unions, with soft caps one pad-class down.
    rowcols2 = {}
    for r in nz:
        for c in cols_of[r]:
            if c not in rowcols2:
                rowcols2[c] = np.flatnonzero(m2[c] & live1)
    CAP3, CAP2 = 3 * P - 1, 2 * P - 1
    mem3 = np.zeros((NCORES, DIMS[2]), bool)
    mem2 = np.zeros((NCORES, DIMS[1]), bool)
    n3 = [0] * NCORES
    n2 = [0] * NCORES
    cnt = [0] * NCORES
    assign = [[] for _ in range(NCORES)]
    for r in nz:
        cs = cols_of[r]
        best, bestcost = None, None
        for k in range(NCORES):
            if cnt[k] >= fs3:
                continue
            new3 = [c for c in cs if not mem3[k, c]]
            new2 = sum(int((~mem2[k, rowcols2[c]]).sum()) for c in new3)
            pen = (10000 if n3[k] + len(new3) > CAP3 else 0) + \
                  (10000 if n2[k] + new2 > CAP2 else 0)
            cost = (pen + new2 + 0.3 * len(new3), n2[k], cnt[k])
            if bestcost is None or cost < bestcost:
                best, bestcost = k, cost
        k = best
        for c in cs:
            if not mem3[k, c]:
                mem2[k, rowcols2[c]] = True
        mem3[k, cs] = True
        n3[k] = int(mem3[k].sum())
        n2[k] = int(mem2[k].sum())
        cnt[k] += 1
        assign[k].append(r)
    # Rows whose m3 row is all-zero produce exactly b3 (and, in prod8
    # mode, exactly 0 on device) — they never touch the device. Each core
    # computes only its nonzero rows, padded to the shared f3 size.
    idxs = []
    k1 = f1 = f2 = f3 = 0
    for k in range(NCORES):
        rows = np.array(sorted(assign[k]), dtype=np.int64)
        S3 = np.flatnonzero(m3[rows].any(axis=0) & live2) if len(rows) \
            else np.zeros(0, np.int64)
        S2 = np.flatnonzero(m2[S3].any(axis=0) & live1)
        S1 = np.flatnonzero(m1[S2].any(axis=0))
        if len(S3) > PACK_MAX or len(S2) > PACK_MAX or len(S1) > PACK_MAX:
            return None
        idxs.append((S1, S2, S3, rows))
        k1, f1 = max(k1, len(S1)), max(f1, len(S2))
        f2, f3 = max(f2, len(S3)), max(f3, len(rows))
    return (_rup(k1), _rup(f1), _rup(f2), min(_rup(f3), fs3)), idxs


def _b2_adjusted(b1, b2, W2, m1, m2):
    """b2 with the constant h1 features' contributions folded in:
    b2_adj[c] = b2[c] + sum_{i: m1 row i empty} W2m[c,i] * tanh(b1[i])."""
    live1 = np.asarray(m1).any(axis=1)
    th1 = np.tanh(np.asarray(b1, np.float32))
    r2, c2 = np.nonzero(np.asarray(m2))
    sel = ~live1[c2]
    b2a = np.asarray(b2, np.float32).copy()
    np.add.at(b2a, r2[sel],
              np.asarray(W2, np.float32)[r2[sel], c2[sel]] * th1[c2[sel]])
    return b2a


def _build_packed(k1, f1, f2, f3=None, rep=None):
    """Single-core-local packed MLP: [k1]->[f1]->[f2]->[512], B=2048.
    Same NEFF on all 8 cores; per-core inputs differ. No collectives.
    rep (env BASS_MLP_REP, default 1) unrolls the compute pipeline for
    device-time measurement via chain-marginal differencing.

    All weights/activations stay SBUF-resident. Work is tiled per single
    512-batch block (one PSUM bank): finer granularity starts the serial
    ScalarE tanh chain ~1.5us earlier and overlaps the ~0.9us DMA-
    completion semaphore latencies across blocks. The fp8 DoubleRow zero
    pad planes of x/h are built on device (memsets off the critical path)
    so the x DMA moves only real bytes. Layer-3 evictions alternate
    DVE/ScalarE; the last block splits into halves across both engines."""
    import concourse.tile as tile
    from concourse import bacc, mybir
    from concourse.bass import DynSlice

    cdt = {
        "fp8": mybir.dt.float8e4,
        "fp16": mybir.dt.float16,
        "bf16": mybir.dt.bfloat16,
        "fp32r": mybir.dt.float32r,
        "fp32": mybir.dt.float32,
    }[DTYPE]
    odt = {"fp32": mybir.dt.float32,
           "prod8": mybir.dt.float8e4}.get(OUT_DT, mybir.dt.float16)
    sw, sx = _scales()
    dscale = [1.0 / (sw * sx), 1.0 / sw, 1.0 / sw]   # PSUM descale per layer
    prod8 = OUT_DT == "prod8"
    if prod8:
        dscale[2] *= OSCALE
    use_dr = DTYPE == "fp8"

    if f3 is None:
        f3 = DIMS[3] // NCORES                 # output rows per core
    KS = [_kpad(k1), _kpad(f1), _kpad(f2)]     # contraction per layer (padded)
    FS = [f1, f2, f3]                          # output features per layer
    BOFF = [0, f1 // P, (f1 + f2) // P]        # bias column offsets

    nc = bacc.Bacc(None, target_bir_lowering=False, debug=False,
                   num_devices=NCORES)

    # xp carries only the REAL k1 rows; the fp8 DoubleRow zero pad planes
    # are memset on device, halving the x DMA (the L1 critical path).
    # All three weight matrices AND the fp32 biases (shipped as raw bytes,
    # fp32-bitcast on device) ride ONE fused DRAM tensor/DMA: the HWDGE
    # descriptor-gen and the DMA transfer engine are single shared
    # resources, so every extra input DMA adds ~2us of serial latency
    # (desc-gen + launch + completion-sem) to the input phase no matter
    # which queue it rides.
    WOFF = []   # per-layer column offset into the fused [P, wcols] tile
    wcols = 0
    for li in range(3):
        WOFF.append(wcols)
        wcols += (KS[li] // P) * FS[li]
    BOFFB = wcols                        # bias bytes offset
    nbias = (f1 + f2 + f3) // P          # fp32 bias columns per partition
    wcols += 4 * nbias
    XOFF = wcols                         # x columns offset ([ko, B] flat)
    wcols += (k1 // P) * B
    wall = nc.dram_tensor("wall", [P, wcols], cdt, kind="ExternalInput")
    out = nc.dram_tensor("out", [f3, B], odt, kind="ExternalOutput")

    with tile.TileContext(nc) as tc:
        with tc.tile_pool(name="per", bufs=1) as per, \
             tc.tile_pool(name="op", bufs=8) as opool, \
             tc.tile_pool(name="ps", bufs=4, space="PSUM") as pspool:

            # ---- persistent SBUF residents ----
            # x/h tiles are sized to their layer's padded contraction; pad
            # planes beyond the real features are zeroed once below, off
            # the critical path (Pool for x, DVE for h — both idle early).
            tcols = XOFF + (KS[0] // P) * B   # + device-side DR pad planes
            wt_all = per.tile([P, tcols], cdt, tag="wall", name="wt_all")
            xt = wt_all[:, XOFF:tcols].rearrange("p (ko n) -> p ko n",
                                                 ko=KS[0] // P)
            wt = [wt_all[:, WOFF[li]:WOFF[li] + (KS[li] // P) * FS[li]]
                  .rearrange("p (ko f) -> p ko f", ko=KS[li] // P)
                  for li in range(3)]
            h = [per.tile([P, KS[li + 1] // P, B], cdt, tag=f"h{li}",
                          name=f"ht{li}") for li in range(2)]
            bt = wt_all[:, BOFFB:BOFFB + 4 * nbias].bitcast(mybir.dt.float32)
            # Input DMAs: all on the sync queue in first-use order — the
            # fused weights, then x per 512-batch block with the bias
            # tucked in after the first block (the shared desc-gen and
            # transfer engines serialize everything anyway, so order is
            # the only lever). DoubleRow pad-plane memsets: x and h2 on
            # Pool, h1 on DVE — each finishes well before its first reader.
            # ONE input DMA carries weights, bias bytes AND x: with the
            # shared desc-gen/launch/completion-sem latencies (~2.2us per
            # DMA chain), a single blob beats chunked x arrivals — the
            # whole input lands before the first chunked x0 would have
            # cleared its own semaphore, and the tanh chain then runs
            # back-to-back with no x-pacing stalls.
            nc.sync.dma_start(wt_all[:, 0:wcols], wall.ap())

            # dummy 1-element tanh: pulls the ACT function-table load into
            # the DMA head instead of delaying the first real eviction
            warm = per.tile([1, 1], mybir.dt.float32, tag="warm", name="warm")
            nc.gpsimd.memset(warm[:], 0.0)
            nc.scalar.activation(warm[:], warm[:],
                                 mybir.ActivationFunctionType.Tanh)


            if KS[0] > k1:
                nc.gpsimd.memset(wt_all[:, XOFF + (k1 // P) * B:tcols], 0.0)
            if KS[1] > FS[0]:
                nc.vector.memset(
                    h[0][:, slice(FS[0] // P, KS[1] // P), :], 0.0)
            if KS[2] > FS[1]:
                nc.gpsimd.memset(
                    h[1][:, slice(FS[1] // P, KS[2] // P), :], 0.0)

            # out-DMA queues: all on sync (idle after the input loads, and
            # HWDGE desc-gen at 625ns beats gpsimd's 1038ns SWDGE). Never
            # scalar — that queue shares the ACT sequencer and a waiting
            # dma_start would head-of-line-block the eviction dispatches.
            oqs = [nc.sync] * 7
            if rep is None:
                rep = int(os.environ.get("BASS_MLP_REP", "1"))
            for _r in range(rep):
                _layers(nc, tc, mybir, DynSlice, opool, pspool, oqs,
                        KS, FS, BOFF, xt, wt, h, bt, out,
                        use_dr, dscale, odt, _r, prod8)

    nc.compile()
    return nc


def _layers(nc, tc, mybir, DynSlice, opool, pspool, oqs,
            KS, FS, BOFF, xt, wt, h, bt, out, use_dr, dscale, odt, _r,
            prod8):
            # Batch granularity per layer: L1 evicts per single 512-block
            # (the ACT tanh chain starts right after the first block's
            # matmul), L2 per 1024-pair (fewer ACT ops once the chain is
            # saturated), L3 per 512-block on DVE+Pool in parallel (off
            # the ACT chain), flushed as two [128,1024] DMAs — out-DMA
            # desc-gens serialize on the shared HWDGE, so fewer is faster.
            def mms(li, pdst, wsl, bstart):
                KO = KS[li] // P
                # L1 matmuls start from a cold (p-state-ramped) PE after
                # each x-chunk wait: issue as two back-to-back halves so
                # only the first rides the slow ramp
                nsub = 2 if li == 0 else 1
                sw = FD // nsub
                ko = 0
                while ko < KO:
                    dr = use_dr and ko + 1 < KO
                    step = 2 if dr else 1
                    pm = (mybir.MatmulPerfMode.DoubleRow if dr else None)
                    for s in range(nsub):
                        src = (xt if li == 0 else h[li - 1])[
                            :, slice(ko, ko + step),
                            DynSlice(bstart + s * sw, sw)]
                        nc.tensor.matmul(
                            pdst[:, DynSlice(s * sw, sw)] if nsub > 1
                            else pdst,
                            wt[li][:, slice(ko, ko + step), wsl], src,
                            perf_mode=pm,
                            start=(ko == 0), stop=(ko + step >= KO))
                    ko += step

            # L1 evicts per single 512-block (the ACT tanh chain starts
            # right after the first block's matmul and stays x-paced),
            # L2 evicts 1024-pairs (fewer ACT ops once the chain runs).
            GRPS = [tuple((b * FD, FD) for b in range(NB)),
                    ((0, 2 * FD), (2 * FD, 2 * FD))]
            for li in range(2):
                for g0, bw in GRPS[li]:
                    gsl = DynSlice(g0, bw)
                    for f in range(FS[li] // P):
                        wsl = DynSlice(f * P, P)
                        # uniform 2-bank slots (one tag) so the pool fits
                        # PSUM exactly; 512-wide users take the low half
                        pfull = pspool.tile([P, 2 * FD], mybir.dt.float32,
                                            tag="ps",
                                            name=f"ps{_r}_{li}_{f}_{g0}")
                        for bb in range(bw // FD):
                            mms(li, pfull[:, DynSlice(bb * FD, FD)], wsl,
                                g0 + bb * FD)
                        nc.scalar.activation(
                            h[li][:, f, gsl], pfull[:, 0:bw],
                            mybir.ActivationFunctionType.Tanh,
                            bias=bt[:, DynSlice(BOFF[li] + f, 1)],
                            scale=dscale[li])

            # L3: per-512-block psum tiles (a shared tile would WAR-stall
            # the next block's matmul behind this block's eviction), then
            # evictions alternating DVE/ACT (GPSIMD cannot read PSUM; ACT
            # is free once the tanh chain ends), flushed as two
            # [128,1024] DMAs — out-DMA desc-gens serialize on the shared
            # HWDGE, so fewer is faster.
            engs = [nc.vector, nc.scalar, nc.scalar, nc.vector]
            for f in range(FS[2] // P):
                wsl = DynSlice(f * P, P)
                bias = bt[:, DynSlice(BOFF[2] + f, 1)]
                ots = [opool.tile([P, 2 * FD], odt, tag="prod",
                                  name=f"o{_r}_{f}_{g}") for g in range(2)]
                for b in range(NB):
                    pfull = pspool.tile([P, 2 * FD], mybir.dt.float32,
                                        tag="ps", name=f"ps{_r}_2_{f}_{b}")
                    psl = pfull[:, 0:FD]
                    mms(2, psl, wsl, b * FD)
                    osl = ots[b // 2][:, DynSlice((b % 2) * FD, FD)]
                    eng = engs[b]
                    if eng is nc.scalar:
                        nc.scalar.activation(
                            osl, psl,
                            mybir.ActivationFunctionType.Identity,
                            bias=0.0 if prod8 else bias, scale=dscale[2])
                    elif prod8:
                        eng.tensor_scalar_mul(osl, psl, dscale[2])
                    else:
                        eng.tensor_scalar(osl, psl, dscale[2], bias,
                                          mybir.AluOpType.mult,
                                          mybir.AluOpType.add)
                    if b % 2 == 1:
                        nc.sync.dma_start(
                            out.ap()[wsl, DynSlice((b - 1) * FD, 2 * FD)],
                            ots[b // 2][:])


def make_in_maps_packed(x, W1, b1, m1, W2, b2, m2, W3, b3, m3, sizes, idxs):
    """Gather per-core packed (and for fp8, pre-scaled) submatrices plus
    the concatenated fp32 bias vector."""
    k1, f1, f2, f3 = sizes
    npdt = _np_cdt()
    sw, sx = _scales()
    x, W1, b1, m1, W2, b2, m2, W3, b3, m3 = (
        np.asarray(a) for a in (x, W1, b1, m1, W2, b2, m2, W3, b3, m3))
    b2a = _b2_adjusted(b1, b2, W2, m1, m2)
    in_maps = []
    for k in range(NCORES):
        S1, S2, S3, rows = idxs[k]
        m = {}
        xk = np.zeros((k1, B), npdt)
        xk[:len(S1)] = (x[:, S1].T * sx).astype(npdt) if sx != 1.0 \
            else x[:, S1].T

        w1 = np.zeros((_kpad(k1), f1), npdt)
        w1[:len(S1), :len(S2)] = (
            (W1[np.ix_(S2, S1)] * m1[np.ix_(S2, S1)]).T * sw)
        w2 = np.zeros((_kpad(f1), f2), npdt)
        w2[:len(S2), :len(S3)] = (
            (W2[np.ix_(S3, S2)] * m2[np.ix_(S3, S2)]).T * sw)
        w3 = np.zeros((_kpad(f2), f3), npdt)
        w3[:len(S3), :len(rows)] = (
            (W3[np.ix_(rows, S3)] * m3[np.ix_(rows, S3)]).T * sw)
        bv = np.zeros(f1 + f2 + f3, np.float32)
        bv[:len(S2)] = b1[S2]
        bv[f1:f1 + len(S3)] = b2a[S3]
        bv[f1 + f2:f1 + f2 + len(rows)] = b3[rows]
        # fused weight tensor: each w [(ko p), f] -> [p, ko*f], the three
        # concatenated along columns (matches the device's WOFF layout),
        # then the fp32 bias bytes (bitcast back to fp32 on device)
        bbytes = np.ascontiguousarray(
            bv.reshape(-1, P).T).view(np.uint8).view(npdt)
        m["wall"] = np.concatenate(
            [w.reshape(-1, P, w.shape[1]).transpose(1, 0, 2).reshape(P, -1)
             for w in (w1, w2, w3)] + [bbytes] +
            [xk.reshape(-1, P, B).transpose(1, 0, 2).reshape(P, -1)], axis=1)
        in_maps.append(m)
    return in_maps


# --------------------------------------------------------------------------
# Dense fallback (Megatron column-parallel, AllGather after layers 1/2)
# --------------------------------------------------------------------------

def _build(l1k=DIMS[0]):
    """Build + schedule the SPMD Bass program (same NEFF on all 8 cores).

    l1k: layer-1 contraction size. DIMS[0] for the dense path; a smaller
    multiple of 512 when the host packs only the K-rows that survive m1
    (per-core), padding with zeros.
    """
    import concourse.tile as tile
    from concourse import bacc, mybir
    from concourse.bass import DynSlice

    cdt = {
        "fp16": mybir.dt.float16,
        "bf16": mybir.dt.bfloat16,
        "fp32r": mybir.dt.float32r,  # rounded fp32; np side is float32
        "fp32": mybir.dt.float32,
    }[_dense_dtype()]
    esz = mybir.dt.size(cdt)

    # Per-layer output-feature shard sizes and weight-panel widths.
    FS = [DIMS[1] // NCORES, DIMS[2] // NCORES, DIMS[3] // NCORES]  # 1024,1024,512
    KS = [l1k, DIMS[1], DIMS[2]]
    if esz == 2:
        # Uniform 64KB/partition weight-panel slots so wpool can double-buffer:
        # the next panel's DMA+mask overlaps the current panel's matmuls.
        FBLK = [1024, 512, 512]
        mck, ibufs, wbufs = MCK, 6, 2
    else:
        FBLK = [1024, 512, 512]      # L2 split into two panels (SBUF)
        mck, ibufs, wbufs = 2, 4, 1

    nc = bacc.Bacc(None, target_bir_lowering=False, debug=False, num_devices=NCORES)

    xT = nc.dram_tensor("xT", [KS[0], B], cdt, kind="ExternalInput")
    wts, mts, bs = [], [], []
    for li in range(3):
        wts.append(nc.dram_tensor(f"w{li + 1}t", [KS[li], FS[li]], cdt,
                                  kind="ExternalInput"))
        mts.append(nc.dram_tensor(f"m{li + 1}t", [KS[li], FS[li]], cdt,
                                  kind="ExternalInput"))
        bs.append(nc.dram_tensor(f"b{li + 1}", [FS[li]], mybir.dt.float32,
                                 kind="ExternalInput"))
    out = nc.dram_tensor("out", [FS[2], B], mybir.dt.float32,
                         kind="ExternalOutput")

    with tile.TileContext(nc) as tc:
        with tc.tile_pool(name="wp", bufs=wbufs) as wpool, \
             tc.tile_pool(name="inp", bufs=ibufs) as ipool, \
             tc.tile_pool(name="mp", bufs=2) as mpool, \
             tc.tile_pool(name="op", bufs=6) as opool, \
             tc.tile_pool(name="bp", bufs=3) as bpool, \
             tc.tile_pool(name="ps", bufs=8, space="PSUM") as pspool, \
             tc.tile_pool(name="dram", bufs=1, space="DRAM") as dram:

            # Per-(layer, b-block) activation tensors so each AllGather covers
            # one 512-batch block and pipelines behind compute.
            h_loc = [[dram.tile([FS[li], FD], cdt, name=f"h{li + 1}_loc{b}")
                      for b in range(NB)] for li in range(2)]
            h_full = [[dram.tile([DIMS[li + 1], FD], cdt, addr_space="Shared",
                                 name=f"h{li + 1}_full{b}")
                       for b in range(NB)] for li in range(2)]

            def layer(li, tanh):
                K, F = KS[li], FS[li]
                KO = K // P
                wt_r = wts[li].ap().rearrange("(ko p) f -> p ko f", p=P)
                mt_r = mts[li].ap().rearrange("(ko p) f -> p ko f", p=P)
                if li == 0:
                    xr = xT.ap().rearrange("(ko p) n -> p ko n", p=P)
                    in_rs = [xr[:, :, DynSlice(b * FD, FD)] for b in range(NB)]
                else:
                    in_rs = [h_full[li - 1][b][:].rearrange(
                        "(ko p) n -> p ko n", p=P) for b in range(NB)]

                btile = bpool.tile([P, F // P], mybir.dt.float32, tag="bias",
                                   name=f"bias{li}")
                nc.sync.dma_start(btile[:], bs[li].ap().rearrange(
                    "(o p) -> p o", p=P))

                fblk = FBLK[li]
                for f0 in range(0, F, fblk):
                    # --- load + mask one weight panel [P, KO, fblk] ---
                    wp = wpool.tile([P, KO, fblk], cdt, tag="wpanel",
                                    name=f"wp{li}_{f0}")
                    # weight/mask loads go on gpsimd/vector DMA queues so the
                    # input-strip stream on the sync queue is never stuck
                    # behind a 16MB panel load
                    for c0 in range(0, KO, mck):
                        csl = slice(c0, c0 + mck)
                        fsl = DynSlice(f0, fblk)
                        nc.gpsimd.dma_start(wp[:, csl, :], wt_r[:, csl, fsl])
                        mtile = mpool.tile([P, mck, fblk], cdt, tag="mchunk",
                                           name=f"m{li}_{f0}_{c0}")
                        nc.gpsimd.dma_start(mtile[:], mt_r[:, csl, fsl])
                        nc.vector.tensor_tensor(wp[:, csl, :], wp[:, csl, :],
                                                mtile[:], mybir.AluOpType.mult)

                    nf = fblk // P
                    for b in range(NB):
                        psums = [pspool.tile([P, FD], mybir.dt.float32,
                                             tag="ps", name=f"ps{li}_{f0}_{b}_{f}")
                                 for f in range(nf)]
                        for c0 in range(0, KO, ICK):
                            it = ipool.tile([P, ICK, FD], cdt, tag="instrip",
                                            name=f"in{li}_{f0}_{b}_{c0}")
                            nc.sync.dma_start(
                                it[:], in_rs[b][:, slice(c0, c0 + ICK), :])
                            for f in range(nf):
                                for ks in range(ICK):
                                    ko = c0 + ks
                                    nc.tensor.matmul(
                                        psums[f][:],
                                        wp[:, ko, DynSlice(f * P, P)],
                                        it[:, ks, :],
                                        start=(ko == 0), stop=(ko == KO - 1))
                        for f in range(nf):
                            fg = f0 + f * P   # feature row offset in shard
                            odt = cdt if li < 2 else mybir.dt.float32
                            ot = opool.tile([P, FD], odt, tag="prod",
                                            name=f"o{li}_{f0}_{b}_{f}")
                            func = (mybir.ActivationFunctionType.Tanh if tanh
                                    else mybir.ActivationFunctionType.Identity)
                            nc.scalar.activation(
                                ot[:], psums[f][:], func,
                                bias=btile[:, DynSlice((f0 // P) + f, 1)])
                            if li < 2:
                                nc.sync.dma_start(
                                    h_loc[li][b][DynSlice(fg, P), :], ot[:])
                            else:
                                nc.sync.dma_start(
                                    out.ap()[DynSlice(fg, P),
                                             DynSlice(b * FD, FD)], ot[:])
                        # fire this b-block's AllGather as soon as the last
                        # panel has written it
                        if li < 2 and f0 == F - fblk:
                            nc.gpsimd.collective_compute(
                                "AllGather",
                                mybir.AluOpType.bypass,
                                replica_groups=[list(range(NCORES))],
                                ins=[h_loc[li][b].opt()],
                                outs=[h_full[li][b].opt()],
                            )

            layer(0, tanh=True)
            layer(1, tanh=True)
            layer(2, tanh=False)

    nc.compile()
    return nc


PACK_K = 512   # packed layer-1 contraction size (dense-path fast variant)


def get_nc(l1k=DIMS[0]):
    key = ("dense", l1k)
    if key not in _cache:
        _cache[key] = _build(l1k)
    return _cache[key]


def get_nc_packed(sizes):
    key = ("packed", sizes, os.environ.get("BASS_MLP_REP", "1"))
    if key not in _cache:
        _cache[key] = _build_packed(*sizes)
    return _cache[key]


def plan_l1k(m1):
    """If m1 is sparse enough that every core's shard of (W1*m1).T touches at
    most PACK_K input dims, return (PACK_K, per-core used-row indices); else
    the dense plan."""
    m1 = np.asarray(m1)
    fs = DIMS[1] // NCORES
    idxs = []
    for k in range(NCORES):
        idx = np.flatnonzero(m1[k * fs:(k + 1) * fs].any(axis=0))
        if len(idx) > PACK_K:
            return DIMS[0], None
        idxs.append(idx)
    return PACK_K, idxs


def make_in_maps(x, W1, b1, m1, W2, b2, m2, W3, b3, m3, idxs=None):
    """Host-side sharding: transpose to [K, F] layouts, cast, slice shards.
    With idxs, layer-1 operands are gathered to the PACK_K used K-rows."""
    x, W1, b1, m1, W2, b2, m2, W3, b3, m3 = (
        np.asarray(a) for a in (x, W1, b1, m1, W2, b2, m2, W3, b3, m3))
    npdt = _np_dense_dt()
    xT = np.ascontiguousarray(x.T).astype(npdt, copy=False)
    Ws = [W1, W2, W3]
    Ms = [m1, m2, m3]
    Bs = [b1, b2, b3]
    in_maps = []
    for k in range(NCORES):
        m = {}
        for li in range(3):
            F = DIMS[li + 1]
            fs = F // NCORES
            sl = slice(k * fs, (k + 1) * fs)
            wt = Ws[li][sl].T
            mt = Ms[li][sl].T
            if li == 0:
                if idxs is None:
                    m["xT"] = xT
                else:
                    idx = idxs[k]
                    xk = np.zeros((PACK_K, B), npdt)
                    xk[:len(idx)] = xT[idx]
                    m["xT"] = xk
                    wk = np.zeros((PACK_K, fs), npdt)
                    wk[:len(idx)] = wt[idx].astype(npdt)
                    mk = np.zeros((PACK_K, fs), npdt)
                    mk[:len(idx)] = mt[idx].astype(npdt)
                    m["w1t"], m["m1t"] = wk, mk
            if f"w{li + 1}t" not in m:
                m[f"w{li + 1}t"] = np.ascontiguousarray(wt).astype(
                    npdt, copy=False)
                m[f"m{li + 1}t"] = np.ascontiguousarray(mt).astype(npdt)
            m[f"b{li + 1}"] = np.ascontiguousarray(Bs[li][sl]).astype(
                np.float32, copy=False)
        in_maps.append(m)
    return in_maps


def prepare(x, W1, b1, m1, W2, b2, m2, W3, b3, m3):
    """Plan, build (cached), and shard: returns (nc, in_maps, rowperm).
    rowperm is the output-row order produced by concatenating the per-core
    results (None for the dense fallback's natural order)."""
    plan = plan_packed(m1, m2, m3)
    if plan is not None:
        sizes, idxs = plan
        nc = get_nc_packed(sizes)
        in_maps = make_in_maps_packed(x, W1, b1, m1, W2, b2, m2, W3, b3, m3,
                                      sizes, idxs)
        rowperm = (sizes[3], [rows for _, _, _, rows in idxs])
    else:
        l1k, idxs = plan_l1k(m1)
        nc = get_nc(l1k)
        in_maps = make_in_maps(x, W1, b1, m1, W2, b2, m2, W3, b3, m3,
                               idxs=idxs)
        rowperm = None
    return nc, in_maps, rowperm


def kernel(x, W1, b1, m1, W2, b2, m2, W3, b3, m3):
    from concourse.bass_utils import run_bass_kernel_spmd

    nc, in_maps, rowperm = prepare(x, W1, b1, m1, W2, b2, m2, W3, b3, m3)
    res = run_bass_kernel_spmd(nc, in_maps, core_ids=list(range(NCORES)))
    if rowperm is not None:
        # scatter each core's real rows; rows the device never computed
        # are pure (adjusted) bias. Constant h2 features (empty m2 rows,
        # h2 = tanh(b2)) were excluded on device — fold their output
        # contribution in here.
        f3, row_lists = rowperm
        b3f = np.asarray(b3, np.float32)
        m1a = np.asarray(m1)
        m2a = np.asarray(m2)
        m3a = np.asarray(m3)
        W3f = np.asarray(W3, np.float32)
        b2a = _b2_adjusted(b1, b2, W2, m1, m2)
        live2 = (m2a & m1a.any(axis=1)[None, :]).any(axis=1)
        r3, c3 = np.nonzero(m3a)
        cm = ~live2[c3]
        fold = np.zeros(DIMS[3], np.float32)
        np.add.at(fold, r3[cm],
                  W3f[r3[cm], c3[cm]] * np.tanh(b2a[c3[cm]]))
        full = np.zeros((DIMS[3], B), np.float32)
        for k in range(NCORES):
            rk = row_lists[k]
            full[rk] = np.asarray(res.results[k]["out"][:len(rk)],
                                  np.float32)
        if OUT_DT == "prod8":
            # device emitted OSCALE * (h2 @ W3masked); add biases here
            full = full / OSCALE + (b3f + fold)[:, None]
        else:
            covered = np.zeros(DIMS[3], bool)
            for rk in row_lists:
                covered[rk] = True
            full[~covered] = b3f[~covered, None]
            full += fold[:, None]
        return np.ascontiguousarray(full.T)
    outT = np.concatenate([res.results[k]["out"] for k in range(NCORES)],
                          axis=0).astype(np.float32, copy=False)
    return np.ascontiguousarray(outT.T)

